# revision 5
# baseline (speedup 1.0000x reference)
"""Window-matmul Trainium2 kernel for nn_BatchDotPred (edge-major variant).

  scores[e] = dot(feat[src_e], feat[dst_e]),  E=2M, N=100k, D=128.

Per-core design (8 cores, edges sharded by dst range of 12500 nodes):
  - src rows are never gathered: edges are grouped into 128-node src windows
    (host sort); per 128-edge subtile the PE computes
    S[e, f] = sum_n onehot[n, e] * tbl_w[n, f]  (onehot host-built, bf16),
    i.e. the stationary is the subtile's one-hot and the moving tensor is the
    window's node-major table block streamed from HBM. This halves SWDGE
    descriptor load - the measured bottleneck (~2.2 ns/row over 4 queues).
  - dst rows: non-transpose SWDGE dma_gather (edge-major [128, g, 128] tiles)
    from the core's private dst chunk - int16-safe by dst-range sharding.
  - DVE multiplies S (read straight from PSUM, f32) with D into a bf16
    product tile, then reduces the feature axis into a persistent [128, NCOL]
    f32 score strip; one DMA writes all scores out at the end.
  - Window overflow (> CAPW edges) spills to 4 quadrant buckets handled by
    classic two-sided gathers (src rows addressed in the table row-major).
"""

import os

os.environ["BY_DEFAULT_DISABLE_SUBTILE_DEPS"] = "1"

import numpy as np

import concourse.bass as bass
import concourse.bacc as bacc
import concourse.tile as tile
import concourse.mybir as mybir
from concourse import bass_utils

BF16 = mybir.dt.np(mybir.dt.bfloat16)
FP8 = mybir.dt.np(mybir.dt.float8e4)

N_CORES = 8
N_NODES = 100000
N_EDGES = 2000000
D = 128

DCHUNK = N_NODES // N_CORES          # 12500 dst nodes per core
DPAD = 12544
WSZ = 96                             # src window size
NW = (N_NODES + WSZ - 1) // WSZ      # 1042
WPS = 16                             # windows per strip
NSTRIP = -(-NW // WPS)               # 66
NWP = NSTRIP * WPS                   # 1056
CAPW = 256                           # slots per window (2 subtiles of 128)
STRIP = WPS * CAPW                   # 4096 slots (= 4 gathers of 1024)
SLOTS_MAIN = NSTRIP * STRIP          # 301056
NSQ = 4
SPC = 512
SPILL = NSQ * SPC                    # 2048
SLOTS = SLOTS_MAIN + SPILL           # 303104
NI = 1024                            # dst gather tile (non-transpose)
NSUB = STRIP // 128                  # 48 subtiles per strip
NCOL = SLOTS // 128                  # 2368 score columns

_programs = {}


def _build_program():
    nc = bacc.Bacc("TRN2", target_bir_lowering=False, debug=False,
                   num_devices=N_CORES, num_swdge_queues=4)
    f32, bf16, i16 = mybir.dt.float32, mybir.dt.bfloat16, mybir.dt.int16
    tbl_ap = nc.dram_tensor("tblnm", [128, NWP * 128], bf16,
                            kind="ExternalInput").ap()
    dch_ap = nc.dram_tensor("dchunk", [DPAD, D], bf16,
                            kind="ExternalInput").ap()
    oh_ap = nc.dram_tensor("onehot", [128, NWP * CAPW],
                           mybir.dt.float8e4,
                           kind="ExternalInput").ap()
    didx_ap = nc.dram_tensor("dstidx", [128, SLOTS_MAIN // 16], i16,
                             kind="ExternalInput").ap()
    spsi_ap = nc.dram_tensor("spsrcidx", [128, SPILL // 16], i16,
                             kind="ExternalInput").ap()
    spdi_ap = nc.dram_tensor("spdstidx", [128, SPILL // 16], i16,
                             kind="ExternalInput").ap()
    out_ap = nc.dram_tensor("scores", [128, NCOL], f32,
                            kind="ExternalOutput").ap()
    # table rows viewed row-major for spill src gathers: row r = p*NWP + w
    srcq = [tbl_ap[24 * q:24 * (q + 1), :].rearrange("p (w f) -> (p w) f", f=D)
            for q in range(NSQ)]

    with tile.TileContext(nc) as tc:
        with tc.tile_pool(name="strips", bufs=2) as strips, \
             tc.tile_pool(name="small", bufs=3) as small, \
             tc.tile_pool(name="single", bufs=1) as single, \
             tc.tile_pool(name="ps1", bufs=3, space="PSUM") as psum1:
            scores = single.tile([128, NCOL], f32)
            qq = [0]

            def issue_gathers(st):
                didx = small.tile([128, STRIP // 16], i16, tag="didx",
                                  bufs=4, name=f"didx{st}")
                nc.sync.dma_start(
                    didx[:],
                    didx_ap[:, st * (STRIP // 16):(st + 1) * (STRIP // 16)])
                dts = []
                for g in range(STRIP // NI):
                    dt = strips.tile([128, NI], bf16, tag=f"d{g}", bufs=4,
                                     name=f"dt{st}_{g}")
                    nc.gpsimd.dma_gather(
                        out_ap=dt[:].rearrange("p (g d) -> p g d", d=D),
                        in_ap=dch_ap[:],
                        idxs_ap=didx[:, g * (NI // 16):(g + 1) * (NI // 16)],
                        num_idxs=NI, num_idxs_reg=NI, elem_size=D,
                        queue_num=qq[0] % 4)
                    qq[0] += 1
                    dts.append(dt)
                return dts

            def process(st, dts):
                tbs = small.tile([128, WPS * 128], bf16, tag="tbs",
                                 name=f"tbs{st}")
                nc.sync.dma_start(
                    tbs[:], tbl_ap[:, st * WPS * 128:(st + 1) * WPS * 128])
                ohs = small.tile([128, WPS * CAPW], mybir.dt.float8e4,
                                 tag="ohs", name=f"ohs{st}")
                nc.sync.dma_start(
                    ohs[:], oh_ap[:, st * WPS * CAPW:(st + 1) * WPS * CAPW])
                for bank in range(NSUB // 4):
                    ps1t = psum1.tile([128, 512], f32, tag="ps1",
                                      name=f"ps1_{st}_{bank}")
                    for k4 in range(4):
                        sub = bank * 4 + k4
                        j, k = divmod(sub, 2)
                        nc.tensor.matmul(
                            ps1t[:, k4 * 128:(k4 + 1) * 128],
                            ohs[0:WSZ, j * CAPW + k * 128:
                                j * CAPW + (k + 1) * 128],
                            tbs[0:WSZ, j * 128:(j + 1) * 128])
                    s_sb = strips.tile([128, 512], bf16, tag="ssb", bufs=3,
                                       name=f"ssb_{st}_{bank}")
                    nc.scalar.copy(s_sb[:], ps1t[:])
                    prod = strips.tile([128, 512], bf16, tag="prod", bufs=3,
                                       name=f"prod_{st}_{bank}")
                    gt, h = divmod(bank, 2)
                    nc.vector.tensor_mul(prod[:], s_sb[:],
                                         dts[gt][:, h * 512:(h + 1) * 512])
                    nc.vector.tensor_reduce(
                        out=scores[:, st * NSUB + bank * 4:
                                   st * NSUB + bank * 4 + 4],
                        in_=prod[:].rearrange("p (g d) -> p g d", d=D),
                        axis=mybir.AxisListType.X,
                        op=mybir.AluOpType.add)

            pend = []
            for st in range(NSTRIP):
                pend.append((st, issue_gathers(st)))
                if len(pend) > 2:
                    process(*pend.pop(0))
            for item in pend:
                process(*item)

            # ---- spill path ----
            spdi = single.tile([128, SPILL // 16], i16)
            nc.sync.dma_start(spdi[:], spdi_ap[:])
            spsi = single.tile([128, SPILL // 16], i16)
            nc.sync.dma_start(spsi[:], spsi_ap[:])
            spd = single.tile([128, SPILL], bf16)
            for g in range(SPILL // NI):
                nc.gpsimd.dma_gather(
                    out_ap=spd[:, g * NI:(g + 1) * NI].rearrange(
                        "p (g d) -> p g d", d=D),
                    in_ap=dch_ap[:],
                    idxs_ap=spdi[:, g * (NI // 16):(g + 1) * (NI // 16)],
                    num_idxs=NI, num_idxs_reg=NI, elem_size=D,
                    queue_num=qq[0] % 4)
                qq[0] += 1
            sps = single.tile([128, SPILL], bf16)
            for sq in range(NSQ):
                nc.gpsimd.dma_gather(
                    out_ap=sps[:, sq * SPC:(sq + 1) * SPC].rearrange(
                        "p (g d) -> p g d", d=D),
                    in_ap=srcq[sq],
                    idxs_ap=spsi[:, sq * (SPC // 16):(sq + 1) * (SPC // 16)],
                    num_idxs=SPC, num_idxs_reg=SPC, elem_size=D,
                    queue_num=qq[0] % 4)
                qq[0] += 1
            spp = single.tile([128, SPILL], bf16)
            nc.vector.tensor_mul(spp[:], sps[:], spd[:])
            nc.vector.tensor_reduce(
                out=scores[:, NSTRIP * NSUB:NSTRIP * NSUB + SPILL // 128],
                in_=spp[:].rearrange("p (g d) -> p g d", d=D),
                axis=mybir.AxisListType.X,
                op=mybir.AluOpType.add)
            nc.sync.dma_start(out_ap[:], scores[:])

    nc.compile()
    return nc


def _get_program():
    if "p" not in _programs:
        _programs["p"] = _build_program()
    return _programs["p"]


def _wrap_idx(idx16: np.ndarray, ni: int) -> np.ndarray:
    """[G*ni] int16 -> [128, G*(ni//16)] wrapped + replicated layout."""
    g = idx16.size // ni
    w = ni // 16
    a = idx16.reshape(g, w, 16).transpose(0, 2, 1)
    a = np.tile(a, (1, 8, 1))
    return np.ascontiguousarray(a.transpose(1, 0, 2).reshape(128, g * w))


def _pack_core(src, dst, eids, base):
    """One core's edges (dst in [base, base+DCHUNK))."""
    w = src // WSZ
    order = np.argsort(w, kind="stable")
    ws = w[order]
    counts = np.bincount(ws, minlength=NW)
    starts = np.zeros(NW, np.int64)
    starts[1:] = np.cumsum(counts)[:-1]

    slot2edge = np.full(SLOTS, -1, np.int64)
    dst_local = np.zeros(SLOTS_MAIN, np.int16)
    spill = []
    for wi in np.nonzero(counts)[0]:
        k = counts[wi]
        s0 = starts[wi]
        take = min(k, CAPW)
        stp, j = divmod(wi, WPS)
        b = stp * STRIP + j * CAPW
        sel = order[s0:s0 + take]
        slot2edge[b:b + take] = sel
        dst_local[b:b + take] = (dst[sel] - base).astype(np.int16)
        if k > take:
            spill.append(order[s0 + take:s0 + k])

    oh = np.zeros((128, NWP * CAPW), FP8)
    sidx = np.nonzero(slot2edge[:SLOTS_MAIN] >= 0)[0]
    stp = sidx // STRIP
    r = sidx % STRIP
    col = (stp * WPS + r // CAPW) * CAPW + r % CAPW
    e = slot2edge[sidx]
    oh[src[e] % WSZ, col] = 1.0

    sp_dst = np.zeros(SPILL, np.int16)
    sp_src = np.zeros(SPILL, np.int16)
    if spill:
        spill = np.concatenate(spill)
        quad = (src[spill] % WSZ) // 24
        for sq in range(NSQ):
            lst = spill[quad == sq]
            if lst.size > SPC:
                raise OverflowError(f"spill bucket {sq}: {lst.size}")
            b = sq * SPC
            slot2edge[SLOTS_MAIN + b:SLOTS_MAIN + b + lst.size] = lst
            sp_dst[b:b + lst.size] = (dst[lst] - base).astype(np.int16)
            sp_src[b:b + lst.size] = (
                ((src[lst] % WSZ) - 24 * sq) * NWP + (src[lst] // WSZ)
            ).astype(np.int16)

    return {
        "onehot": oh,
        "dstidx": _wrap_idx(dst_local, NI),
        "spdstidx": _wrap_idx(sp_dst, NI),
        "spsrcidx": _wrap_idx(sp_src, SPC),
    }, slot2edge, eids


def _prep_shared(feat):
    feat_bf = feat.astype(BF16)
    t = np.zeros((NWP * WSZ, D), BF16)
    t[:N_NODES] = feat_bf
    t = t.reshape(NWP, WSZ, D)
    full = np.zeros((NWP, 128, D), BF16)
    full[:, :WSZ] = t
    tblnm = np.ascontiguousarray(
        full.transpose(1, 0, 2).reshape(128, NWP * 128))
    chunks = []
    for c in range(N_CORES):
        ch = np.zeros((DPAD, D), BF16)
        ch[:DCHUNK] = feat_bf[c * DCHUNK:(c + 1) * DCHUNK]
        chunks.append(ch)
    return tblnm, chunks


def _run(edges: np.ndarray, feat: np.ndarray, trace: bool = False):
    edges = np.asarray(edges)
    feat = np.ascontiguousarray(np.asarray(feat, dtype=np.float32))
    assert edges.shape == (N_EDGES, 2) and feat.shape == (N_NODES, D)
    src = edges[:, 0].astype(np.int64)
    dst = edges[:, 1].astype(np.int64)
    tblnm, chunks = _prep_shared(feat)

    core_of = dst // DCHUNK
    in_maps, slot_maps, eid_maps = [], [], []
    for c in range(N_CORES):
        sel = np.nonzero(core_of == c)[0]
        m, s2e, eids = _pack_core(src[sel], dst[sel], sel, c * DCHUNK)
        m["tblnm"] = tblnm
        m["dchunk"] = chunks[c]
        in_maps.append(m)
        slot_maps.append(s2e)
        eid_maps.append(eids)

    nc = _get_program()
    res = bass_utils.run_bass_kernel_spmd(
        nc, in_maps, core_ids=list(range(N_CORES)), trace=trace)

    out = np.zeros(N_EDGES, np.float32)
    for c in range(N_CORES):
        # slot s -> scores[s % 128, s // 128]
        flat = res.results[c]["scores"].T.reshape(SLOTS)
        s2e = slot_maps[c]
        valid = s2e >= 0
        out[eid_maps[c][s2e[valid]]] = flat[valid]
    return out[:, None], res


def kernel(edges: np.ndarray, feat: np.ndarray) -> np.ndarray:
    out, _ = _run(edges, feat, trace=False)
    return out
